# revision 12
# baseline (speedup 1.0000x reference)
"""Trainium2 Bass kernel for nn_AttentionBlock (sigmoid-gated attention block).

Strategy: data-parallel over batch B=8 across 8 NeuronCores (1 sample/core).
Per core (x: [C=32, F=32, T=4000], padded to 4096 on host):
  setup:  cast-load ALL of x into SBUF as bf16 ([128=(f4,c), fg, t], 64KB/p).
  pass 1: 1x1-conv+BN+PReLU for q, k (layout [d', t]) and v (layout [t, d']),
          BN folded into conv weights/bias on the host. d' = f*16 + ch,
          f = fg*4 + f4. q/k/v stored bf16.
  pass 2: per t-block of 512: mask_st = sigmoid(k.T q) in 128-row s-chunks
          (flash-style, mask never touches HBM, bf16); out' = sum_s v_s.T
          mask_st accumulated in fp32 PSUM in [d', t] layout; enc conv
          (f32r) + BN + PReLU + residual(bf16 x); store.
The last s-chunk contracts only 32 rows (t in [4000,4096) is padding).
"""
import math

import numpy as np

import concourse.bass as bass
import concourse.mybir as mybir
from concourse import bacc
from concourse import bass_utils
from concourse.tile import TileContext

C, F, T = 32, 32, 4000
CH = 16
D = 512
TB = 512
NTB = 8          # t-blocks (last is ragged: 416 real columns)
NSCH = 32        # s-chunks of 128 (4096 padded)
TPAD = NSCH * 128
EPS = 1e-5

F32 = mybir.dt.float32
F32R = mybir.dt.float32r
BF16 = mybir.dt.bfloat16
AF = mybir.ActivationFunctionType
OP = mybir.AluOpType

TRACE = False
LAST_RESULTS = None


def _build(alpha_q, alpha_k, alpha_v, alpha_e):
    nc = bacc.Bacc("TRN2", target_bir_lowering=False, debug=False)

    x = nc.dram_tensor("x", [C, F, TPAD], F32, kind="ExternalInput").ap()
    wqA = nc.dram_tensor("wqA", [128, 128], F32, kind="ExternalInput").ap()
    wqB = nc.dram_tensor("wqB", [128, 128], F32, kind="ExternalInput").ap()
    wkA = nc.dram_tensor("wkA", [128, 128], F32, kind="ExternalInput").ap()
    wkB = nc.dram_tensor("wkB", [128, 128], F32, kind="ExternalInput").ap()
    wv = nc.dram_tensor("wv", [128, 64], F32, kind="ExternalInput").ap()
    weH = nc.dram_tensor("weH", [128, 128], F32, kind="ExternalInput").ap()
    bq = nc.dram_tensor("bq", [128, 1], F32, kind="ExternalInput").ap()
    bk = nc.dram_tensor("bk", [128, 1], F32, kind="ExternalInput").ap()
    be = nc.dram_tensor("be", [128, 1], F32, kind="ExternalInput").ap()
    bvb = nc.dram_tensor("bvb", [128, 512], F32, kind="ExternalInput").ap()
    y = nc.dram_tensor("y", [C, F, T], F32, kind="ExternalOutput").ap()

    # flat APs for partition p = f4*32 + c at f = fg*4 + f4 (split-partition
    # rearrange APs scramble/crash the HW DMA path; manual APs are exact)
    def x_ap(fg):
        return bass.AP(tensor=x.tensor, offset=fg * 4 * TPAD,
                       ap=[[TPAD, 4], [F * TPAD, 32], [1, TPAD]])

    def y_ap(fg, t0, tw):
        return bass.AP(tensor=y.tensor, offset=fg * 4 * T + t0,
                       ap=[[T, 4], [F * T, 32], [1, tw]])

    with TileContext(nc) as tc:
        with (
            tc.tile_pool(name="persist", bufs=1) as persist,
            tc.tile_pool(name="sapool", bufs=2) as sapool,
            tc.tile_pool(name="mpool", bufs=4) as mpool,
            tc.tile_pool(name="opool", bufs=4) as opool,
            tc.tile_pool(name="pacc", bufs=1, space="PSUM") as pacc,
            tc.tile_pool(name="pmask", bufs=2, space="PSUM") as pmask,
            tc.tile_pool(name="pmisc", bufs=2, space="PSUM") as pmisc,
        ):
            # resident tensors: x (bf16, one tile per fg), q/k in [d', t],
            # v in [t, d']
            xfull = [persist.tile([128, TPAD], BF16, tag=f"xfull{fg}",
                                  name=f"xfull{fg}") for fg in range(8)]
            qT = persist.tile([128, 4, TPAD], BF16, tag="qT")
            kT = persist.tile([128, 4, TPAD], BF16, tag="kT")
            vS = persist.tile([128, NSCH, D], BF16, tag="vS")

            wqA_sb = persist.tile([128, 128], BF16, tag="wqA")
            wqB_sb = persist.tile([128, 128], BF16, tag="wqB")
            wkA_sb = persist.tile([128, 128], BF16, tag="wkA")
            wkB_sb = persist.tile([128, 128], BF16, tag="wkB")
            wv_sb = persist.tile([128, 64], BF16, tag="wv")
            weH_sb = persist.tile([128, 128], F32R, tag="weH")
            nc.gpsimd.dma_start(out=wqA_sb, in_=wqA)
            nc.gpsimd.dma_start(out=wqB_sb, in_=wqB)
            nc.gpsimd.dma_start(out=wkA_sb, in_=wkA)
            nc.gpsimd.dma_start(out=wkB_sb, in_=wkB)
            nc.gpsimd.dma_start(out=wv_sb, in_=wv)
            nc.gpsimd.dma_start(out=weH_sb, in_=weH)
            bq_sb = persist.tile([128, 1], F32, tag="bq")
            bk_sb = persist.tile([128, 1], F32, tag="bk")
            be_sb = persist.tile([128, 1], F32, tag="be")
            bvb_sb = persist.tile([128, 512], F32, tag="bvb")
            nc.sync.dma_start(out=bq_sb, in_=bq)
            nc.sync.dma_start(out=bk_sb, in_=bk)
            nc.sync.dma_start(out=be_sb, in_=be)
            nc.sync.dma_start(out=bvb_sb, in_=bvb)

            # load all of x (cast f32 -> bf16), one DMA per fg
            for fg in range(8):
                nc.gpsimd.dma_start(out=xfull[fg], in_=x_ap(fg))

            # ---------------- pass 1: q, k, v preproc ----------------
            for tb in range(NTB):
                t0 = tb * TB

                # q/k: psum [128=(h,f4,ch), t] per d'-chunk g, fg = 2g+h
                for dst, wA, wB, bias, alpha in (
                    (kT, wkA_sb, wkB_sb, bk_sb, alpha_k),
                    (qT, wqA_sb, wqB_sb, bq_sb, alpha_q),
                ):
                    for g in range(4):
                        pp = pmisc.tile([128, TB], F32, tag="mm", name="pp")
                        nc.tensor.matmul(pp, wA, xfull[2 * g][:, t0:t0 + TB],
                                         start=True, stop=False)
                        nc.tensor.matmul(pp, wB, xfull[2 * g + 1][:, t0:t0 + TB],
                                         start=False, stop=True)
                        ds = dst[:, g, t0:t0 + TB]
                        nc.scalar.activation(ds, pp, AF.Identity,
                                             bias=bias[:, 0:1], scale=1.0)
                        nc.vector.scalar_tensor_tensor(
                            out=ds, in0=ds, scalar=alpha, in1=ds,
                            op0=OP.mult, op1=OP.max)

                # v: psum [128=t, 512=d'], x slice as stationary
                for i4 in range(4):
                    sch = tb * 4 + i4
                    vp = pmisc.tile([128, 512], F32, tag="mm")
                    for fg in range(8):
                        nc.tensor.matmul(
                            vp[:, fg * 64:(fg + 1) * 64],
                            xfull[fg][:, t0 + i4 * 128:t0 + (i4 + 1) * 128],
                            wv_sb, start=True, stop=True)
                    vs = vS[:, sch, :]
                    nc.vector.tensor_add(vs, vp, bvb_sb)
                    nc.vector.scalar_tensor_tensor(
                        out=vs, in0=vs, scalar=alpha_v, in1=vs,
                        op0=OP.mult, op1=OP.max)

            # ---------------- pass 2: attention + enc ----------------
            # enc for t-block tb-1 is emitted after stage-1/2 of t-block tb,
            # so its PE work fills the sigmoid-wait bubbles of the next block.
            def enc_block(sa, tb):
                t0 = tb * TB
                tw = min(TB, T - t0)
                for fg in range(8):
                    g, h = fg // 2, fg % 2
                    ep = pmisc.tile([128, TB], F32, tag="mm", name="ep")
                    nc.tensor.matmul(
                        ep, weH_sb[64 * h:64 * (h + 1), :],
                        sa[64 * h:64 * (h + 1), g, :], start=True, stop=True)
                    es = opool.tile([128, TB], F32, tag="eb", name="es")
                    nc.scalar.activation(es, ep, AF.Identity,
                                         bias=be_sb[:, 0:1], scale=1.0)
                    nc.vector.scalar_tensor_tensor(
                        out=es, in0=es, scalar=alpha_e, in1=es,
                        op0=OP.mult, op1=OP.max)
                    nc.vector.tensor_add(es, es, xfull[fg][:, t0:t0 + TB])
                    nc.sync.dma_start(out=y_ap(fg, t0, tw), in_=es[:, 0:tw])

            pending = None  # (sa, tb) awaiting enc
            for tb in range(NTB):
                t0 = tb * TB

                acc = [pacc.tile([128, TB], F32, tag=f"acc{g}", name=f"acc{g}")
                       for g in range(4)]
                for sch in range(NSCH):
                    mp = pmask.tile([128, TB], F32, tag="mp")
                    for g in range(4):
                        nc.tensor.matmul(
                            mp, kT[:, g, sch * 128:(sch + 1) * 128],
                            qT[:, g, t0:t0 + TB], start=(g == 0), stop=(g == 3))
                    ms = mpool.tile([128, TB], BF16, tag="ms")
                    nc.scalar.activation(ms, mp, AF.Sigmoid)
                    kk = 128 - (TPAD - T) if sch == NSCH - 1 else 128
                    for g in range(4):
                        nc.tensor.matmul(
                            acc[g], vS[0:kk, sch, g * 128:(g + 1) * 128],
                            ms[0:kk, :], start=(sch == 0), stop=(sch == NSCH - 1))

                # psum -> sbuf (f32r) for enc conv
                sa = sapool.tile([128, 4, TB], F32R, tag="sa")
                for g in range(4):
                    nc.vector.tensor_copy(sa[:, g, :], acc[g])

                if pending is not None:
                    enc_block(*pending)
                pending = (sa, tb)
            enc_block(*pending)

    nc.finalize()
    return nc


def _host_prep(inputs):
    """Fold BN into conv weights/bias; build block-diagonal weight layouts."""
    qkv_W = np.asarray(inputs["qkv_W"], np.float64)      # [3, 16, 32]
    qkv_b = np.asarray(inputs["qkv_b"], np.float64)      # [3, 16]
    qkv_gamma = np.asarray(inputs["qkv_gamma"], np.float64)
    qkv_beta = np.asarray(inputs["qkv_beta"], np.float64)
    qkv_mean = np.asarray(inputs["qkv_mean"], np.float64)
    qkv_var = np.asarray(inputs["qkv_var"], np.float64)
    qkv_alpha = np.asarray(inputs["qkv_alpha"], np.float64)  # [3]
    enc_W = np.asarray(inputs["enc_W"], np.float64)      # [32, 16]
    enc_b = np.asarray(inputs["enc_b"], np.float64)      # [32]
    enc_gamma = np.asarray(inputs["enc_gamma"], np.float64)
    enc_beta = np.asarray(inputs["enc_beta"], np.float64)
    enc_mean = np.asarray(inputs["enc_mean"], np.float64)
    enc_var = np.asarray(inputs["enc_var"], np.float64)
    enc_alpha = float(np.asarray(inputs["enc_alpha"]))

    inv = qkv_gamma / np.sqrt(qkv_var + EPS)             # [3, 16]
    W_eff = qkv_W * inv[:, :, None]                      # [3, 16, 32]
    b_eff = (qkv_b - qkv_mean) * inv + qkv_beta          # [3, 16]
    scale = math.sqrt(D)
    W_eff[1] /= scale
    b_eff[1] /= scale

    einv = enc_gamma / np.sqrt(enc_var + EPS)            # [32]
    We_eff = enc_W * einv[:, None]                       # [32, 16]
    be_eff = (enc_b - enc_mean) * einv + enc_beta        # [32]

    def blk(Wn):  # [16, 32] -> [128, 64] block-diag over f4
        out = np.zeros((128, 64), np.float32)
        for f4 in range(4):
            out[f4 * 32:(f4 + 1) * 32, f4 * 16:(f4 + 1) * 16] = Wn.T
        return out

    def blkAB(Wn):
        b = blk(Wn)
        A = np.zeros((128, 128), np.float32)
        Bm = np.zeros((128, 128), np.float32)
        A[:, 0:64] = b
        Bm[:, 64:128] = b
        return A, Bm

    wqA, wqB = blkAB(W_eff[0])
    wkA, wkB = blkAB(W_eff[1])
    wv = blk(W_eff[2])

    we_blk = np.zeros((64, 128), np.float32)             # [(f4,ch), (f4,o)]
    for f4 in range(4):
        we_blk[f4 * 16:(f4 + 1) * 16, f4 * 32:(f4 + 1) * 32] = We_eff.T
    weH = np.concatenate([we_blk, we_blk], axis=0)       # [128, 128]

    p = np.arange(128)
    bq = b_eff[0][p % 16].astype(np.float32)[:, None]    # [128, 1]
    bk = b_eff[1][p % 16].astype(np.float32)[:, None]
    be = be_eff[p % 32].astype(np.float32)[:, None]
    cols = np.arange(512)
    bvb = np.broadcast_to(
        b_eff[2][cols % 16].astype(np.float32), (128, 512)).copy()

    alphas = (float(qkv_alpha[0]), float(qkv_alpha[1]),
              float(qkv_alpha[2]), enc_alpha)
    for a in alphas:
        assert 0.0 <= a <= 1.0
    weights = {
        "wqA": wqA, "wqB": wqB, "wkA": wkA, "wkB": wkB, "wv": wv,
        "weH": weH, "bq": bq, "bk": bk, "be": be, "bvb": bvb,
    }
    return weights, alphas


def kernel(**inputs):
    global LAST_RESULTS
    x = np.ascontiguousarray(np.asarray(inputs["x"], np.float32))  # [8,32,32,4000]
    B = x.shape[0]
    x = np.pad(x, ((0, 0), (0, 0), (0, 0), (0, TPAD - T)))
    weights, alphas = _host_prep(inputs)
    nc = _build(*alphas)
    in_maps = [dict(weights, x=x[b]) for b in range(B)]
    res = bass_utils.run_bass_kernel_spmd(
        nc, in_maps, core_ids=list(range(B)), trace=TRACE)
    LAST_RESULTS = res
    out = np.stack([res.results[b]["y"] for b in range(B)], axis=0)
    return out.astype(np.float32)


# revision 13
# speedup vs baseline: 1.1825x; 1.1825x over previous
"""Trainium2 Bass kernel for nn_AttentionBlock (sigmoid-gated attention block).

Strategy: data-parallel over batch B=8 across 8 NeuronCores (1 sample/core).
Per core (x: [C=32, F=32, T=4000], padded to 4096 on host):
  setup:  cast-load ALL of x into SBUF as bf16 ([128=(f4,c), fg, t], 64KB/p).
  pass 1: 1x1-conv+BN+PReLU for q, k (layout [d', t]) and v (layout [t, d']),
          BN folded into conv weights/bias on the host. d' = f*16 + ch,
          f = fg*4 + f4. q/k/v stored bf16.
  pass 2: per t-block of 512: mask_st = sigmoid(k.T q) in 128-row s-chunks
          (flash-style, mask never touches HBM, bf16); out' = sum_s v_s.T
          mask_st accumulated in fp32 PSUM in [d', t] layout; enc conv
          (f32r) + BN + PReLU + residual(bf16 x); store.
The last s-chunk contracts only 32 rows (t in [4000,4096) is padding).
"""
import math

import numpy as np

import concourse.bass as bass
import concourse.mybir as mybir
from concourse import bacc
from concourse import bass_utils
from concourse.tile import TileContext

C, F, T = 32, 32, 4000
CH = 16
D = 512
TB = 512
NTB = 8          # t-blocks (last is ragged: 416 real columns)
NSCH = 32        # s-chunks of 128 (4096 padded)
TPAD = NSCH * 128
EPS = 1e-5

F32 = mybir.dt.float32
F32R = mybir.dt.float32r
BF16 = mybir.dt.bfloat16
AF = mybir.ActivationFunctionType
OP = mybir.AluOpType

TRACE = False
LAST_RESULTS = None


def _build(alpha_q, alpha_k, alpha_v, alpha_e):
    nc = bacc.Bacc("TRN2", target_bir_lowering=False, debug=False)

    x = nc.dram_tensor("x", [C, F, TPAD], F32, kind="ExternalInput").ap()
    wqA = nc.dram_tensor("wqA", [128, 128], F32, kind="ExternalInput").ap()
    wqB = nc.dram_tensor("wqB", [128, 128], F32, kind="ExternalInput").ap()
    wkA = nc.dram_tensor("wkA", [128, 128], F32, kind="ExternalInput").ap()
    wkB = nc.dram_tensor("wkB", [128, 128], F32, kind="ExternalInput").ap()
    wv = nc.dram_tensor("wv", [128, 64], F32, kind="ExternalInput").ap()
    weH = nc.dram_tensor("weH", [128, 128], F32, kind="ExternalInput").ap()
    bq = nc.dram_tensor("bq", [128, 1], F32, kind="ExternalInput").ap()
    bk = nc.dram_tensor("bk", [128, 1], F32, kind="ExternalInput").ap()
    be = nc.dram_tensor("be", [128, 1], F32, kind="ExternalInput").ap()
    bvb = nc.dram_tensor("bvb", [128, 512], F32, kind="ExternalInput").ap()
    y = nc.dram_tensor("y", [C, F, T], F32, kind="ExternalOutput").ap()

    # flat APs for partition p = f4*32 + c at f = fg*4 + f4 (split-partition
    # rearrange APs scramble/crash the HW DMA path; manual APs are exact)
    def x_ap(fg):
        return bass.AP(tensor=x.tensor, offset=fg * 4 * TPAD,
                       ap=[[TPAD, 4], [F * TPAD, 32], [1, TPAD]])

    def y_ap(fg, t0, tw):
        return bass.AP(tensor=y.tensor, offset=fg * 4 * T + t0,
                       ap=[[T, 4], [F * T, 32], [1, tw]])

    with TileContext(nc) as tc:
        with (
            tc.tile_pool(name="persist", bufs=1) as persist,
            tc.tile_pool(name="sapool", bufs=2) as sapool,
            tc.tile_pool(name="mpool", bufs=4) as mpool,
            tc.tile_pool(name="opool", bufs=4) as opool,
            tc.tile_pool(name="pacc", bufs=1, space="PSUM") as pacc,
            tc.tile_pool(name="pmask", bufs=2, space="PSUM") as pmask,
            tc.tile_pool(name="pmisc", bufs=2, space="PSUM") as pmisc,
        ):
            # resident tensors: x (bf16, one tile per fg), q/k in [d', t],
            # v in [t, d']
            xfull = [persist.tile([128, TPAD], BF16, tag=f"xfull{fg}",
                                  name=f"xfull{fg}") for fg in range(8)]
            qT = persist.tile([128, 4, TPAD], BF16, tag="qT")
            kT = persist.tile([128, 4, TPAD], BF16, tag="kT")
            vS = persist.tile([128, NSCH, D], BF16, tag="vS")

            wqA_sb = persist.tile([128, 128], BF16, tag="wqA")
            wqB_sb = persist.tile([128, 128], BF16, tag="wqB")
            wkA_sb = persist.tile([128, 128], BF16, tag="wkA")
            wkB_sb = persist.tile([128, 128], BF16, tag="wkB")
            wv_sb = persist.tile([128, 64], BF16, tag="wv")
            weH_sb = persist.tile([128, 128], F32R, tag="weH")
            nc.gpsimd.dma_start(out=wqA_sb, in_=wqA)
            nc.gpsimd.dma_start(out=wqB_sb, in_=wqB)
            nc.gpsimd.dma_start(out=wkA_sb, in_=wkA)
            nc.gpsimd.dma_start(out=wkB_sb, in_=wkB)
            nc.gpsimd.dma_start(out=wv_sb, in_=wv)
            nc.gpsimd.dma_start(out=weH_sb, in_=weH)
            bq_sb = persist.tile([128, 1], F32, tag="bq")
            bk_sb = persist.tile([128, 1], F32, tag="bk")
            be_sb = persist.tile([128, 1], F32, tag="be")
            bvb_sb = persist.tile([128, 512], F32, tag="bvb")
            nc.sync.dma_start(out=bq_sb, in_=bq)
            nc.sync.dma_start(out=bk_sb, in_=bk)
            nc.sync.dma_start(out=be_sb, in_=be)
            nc.sync.dma_start(out=bvb_sb, in_=bvb)

            # load all of x (cast f32 -> bf16), one DMA per fg
            for fg in range(8):
                nc.gpsimd.dma_start(out=xfull[fg], in_=x_ap(fg))

            # ---------------- pass 1: q, k, v preproc ----------------
            for tb in range(NTB):
                t0 = tb * TB

                # q/k: psum [128=(h,f4,ch), t] per d'-chunk g, fg = 2g+h
                for dst, wA, wB, bias, alpha in (
                    (kT, wkA_sb, wkB_sb, bk_sb, alpha_k),
                    (qT, wqA_sb, wqB_sb, bq_sb, alpha_q),
                ):
                    for g in range(4):
                        pp = pmisc.tile([128, TB], F32, tag="mm", name="pp")
                        nc.tensor.matmul(pp, wA, xfull[2 * g][:, t0:t0 + TB],
                                         start=True, stop=False)
                        nc.tensor.matmul(pp, wB, xfull[2 * g + 1][:, t0:t0 + TB],
                                         start=False, stop=True)
                        ds = dst[:, g, t0:t0 + TB]
                        nc.scalar.activation(ds, pp, AF.Identity,
                                             bias=bias[:, 0:1], scale=1.0)
                        nc.vector.scalar_tensor_tensor(
                            out=ds, in0=ds, scalar=alpha, in1=ds,
                            op0=OP.mult, op1=OP.max)

                # v: psum [128=t, 512=d'], x slice as stationary
                for i4 in range(4):
                    sch = tb * 4 + i4
                    vp = pmisc.tile([128, 512], F32, tag="mm")
                    for fg in range(8):
                        nc.tensor.matmul(
                            vp[:, fg * 64:(fg + 1) * 64],
                            xfull[fg][:, t0 + i4 * 128:t0 + (i4 + 1) * 128],
                            wv_sb, start=True, stop=True)
                    vs = vS[:, sch, :]
                    nc.vector.tensor_add(vs, vp, bvb_sb)
                    nc.vector.scalar_tensor_tensor(
                        out=vs, in0=vs, scalar=alpha_v, in1=vs,
                        op0=OP.mult, op1=OP.max)

            # ---------------- pass 2: attention + enc ----------------
            def enc_block(sa, tb):
                t0 = tb * TB
                tw = min(TB, T - t0)
                for fg in range(8):
                    g, h = fg // 2, fg % 2
                    ep = pmisc.tile([128, TB], F32, tag="mm", name="ep")
                    nc.tensor.matmul(
                        ep, weH_sb[64 * h:64 * (h + 1), :],
                        sa[64 * h:64 * (h + 1), g, :], start=True, stop=True)
                    es = opool.tile([128, TB], F32, tag="eb", name="es")
                    nc.scalar.activation(es, ep, AF.Identity,
                                         bias=be_sb[:, 0:1], scale=1.0)
                    nc.vector.scalar_tensor_tensor(
                        out=es, in0=es, scalar=alpha_e, in1=es,
                        op0=OP.mult, op1=OP.max)
                    nc.vector.tensor_add(es, es, xfull[fg][:, t0:t0 + TB])
                    nc.sync.dma_start(out=y_ap(fg, t0, tw), in_=es[:, 0:tw])

            for tb in range(NTB):
                t0 = tb * TB

                acc = [pacc.tile([128, TB], F32, tag=f"acc{g}", name=f"acc{g}")
                       for g in range(4)]
                for sch in range(NSCH):
                    mp = pmask.tile([128, TB], F32, tag="mp")
                    for g in range(4):
                        nc.tensor.matmul(
                            mp, kT[:, g, sch * 128:(sch + 1) * 128],
                            qT[:, g, t0:t0 + TB], start=(g == 0), stop=(g == 3))
                    ms = mpool.tile([128, TB], BF16, tag="ms")
                    nc.scalar.activation(ms, mp, AF.Sigmoid)
                    kk = 128 - (TPAD - T) if sch == NSCH - 1 else 128
                    for g in range(4):
                        nc.tensor.matmul(
                            acc[g], vS[0:kk, sch, g * 128:(g + 1) * 128],
                            ms[0:kk, :], start=(sch == 0), stop=(sch == NSCH - 1))

                # psum -> sbuf (f32r) for enc conv
                sa = sapool.tile([128, 4, TB], F32R, tag="sa")
                for g in range(4):
                    nc.vector.tensor_copy(sa[:, g, :], acc[g])

                enc_block(sa, tb)

    nc.finalize()
    return nc


def _host_prep(inputs):
    """Fold BN into conv weights/bias; build block-diagonal weight layouts."""
    qkv_W = np.asarray(inputs["qkv_W"], np.float64)      # [3, 16, 32]
    qkv_b = np.asarray(inputs["qkv_b"], np.float64)      # [3, 16]
    qkv_gamma = np.asarray(inputs["qkv_gamma"], np.float64)
    qkv_beta = np.asarray(inputs["qkv_beta"], np.float64)
    qkv_mean = np.asarray(inputs["qkv_mean"], np.float64)
    qkv_var = np.asarray(inputs["qkv_var"], np.float64)
    qkv_alpha = np.asarray(inputs["qkv_alpha"], np.float64)  # [3]
    enc_W = np.asarray(inputs["enc_W"], np.float64)      # [32, 16]
    enc_b = np.asarray(inputs["enc_b"], np.float64)      # [32]
    enc_gamma = np.asarray(inputs["enc_gamma"], np.float64)
    enc_beta = np.asarray(inputs["enc_beta"], np.float64)
    enc_mean = np.asarray(inputs["enc_mean"], np.float64)
    enc_var = np.asarray(inputs["enc_var"], np.float64)
    enc_alpha = float(np.asarray(inputs["enc_alpha"]))

    inv = qkv_gamma / np.sqrt(qkv_var + EPS)             # [3, 16]
    W_eff = qkv_W * inv[:, :, None]                      # [3, 16, 32]
    b_eff = (qkv_b - qkv_mean) * inv + qkv_beta          # [3, 16]
    scale = math.sqrt(D)
    W_eff[1] /= scale
    b_eff[1] /= scale

    einv = enc_gamma / np.sqrt(enc_var + EPS)            # [32]
    We_eff = enc_W * einv[:, None]                       # [32, 16]
    be_eff = (enc_b - enc_mean) * einv + enc_beta        # [32]

    def blk(Wn):  # [16, 32] -> [128, 64] block-diag over f4
        out = np.zeros((128, 64), np.float32)
        for f4 in range(4):
            out[f4 * 32:(f4 + 1) * 32, f4 * 16:(f4 + 1) * 16] = Wn.T
        return out

    def blkAB(Wn):
        b = blk(Wn)
        A = np.zeros((128, 128), np.float32)
        Bm = np.zeros((128, 128), np.float32)
        A[:, 0:64] = b
        Bm[:, 64:128] = b
        return A, Bm

    wqA, wqB = blkAB(W_eff[0])
    wkA, wkB = blkAB(W_eff[1])
    wv = blk(W_eff[2])

    we_blk = np.zeros((64, 128), np.float32)             # [(f4,ch), (f4,o)]
    for f4 in range(4):
        we_blk[f4 * 16:(f4 + 1) * 16, f4 * 32:(f4 + 1) * 32] = We_eff.T
    weH = np.concatenate([we_blk, we_blk], axis=0)       # [128, 128]

    p = np.arange(128)
    bq = b_eff[0][p % 16].astype(np.float32)[:, None]    # [128, 1]
    bk = b_eff[1][p % 16].astype(np.float32)[:, None]
    be = be_eff[p % 32].astype(np.float32)[:, None]
    cols = np.arange(512)
    bvb = np.broadcast_to(
        b_eff[2][cols % 16].astype(np.float32), (128, 512)).copy()

    alphas = (float(qkv_alpha[0]), float(qkv_alpha[1]),
              float(qkv_alpha[2]), enc_alpha)
    for a in alphas:
        assert 0.0 <= a <= 1.0
    weights = {
        "wqA": wqA, "wqB": wqB, "wkA": wkA, "wkB": wkB, "wv": wv,
        "weH": weH, "bq": bq, "bk": bk, "be": be, "bvb": bvb,
    }
    return weights, alphas


def kernel(**inputs):
    global LAST_RESULTS
    x = np.ascontiguousarray(np.asarray(inputs["x"], np.float32))  # [8,32,32,4000]
    B = x.shape[0]
    x = np.pad(x, ((0, 0), (0, 0), (0, 0), (0, TPAD - T)))
    weights, alphas = _host_prep(inputs)
    nc = _build(*alphas)
    in_maps = [dict(weights, x=x[b]) for b in range(B)]
    res = bass_utils.run_bass_kernel_spmd(
        nc, in_maps, core_ids=list(range(B)), trace=TRACE)
    LAST_RESULTS = res
    out = np.stack([res.results[b]["y"] for b in range(B)], axis=0)
    return out.astype(np.float32)


# revision 14
# speedup vs baseline: 1.2124x; 1.0253x over previous
"""Trainium2 Bass kernel for nn_AttentionBlock (sigmoid-gated attention block).

Strategy: data-parallel over batch B=8 across 8 NeuronCores (1 sample/core).
Per core (x: [C=32, F=32, T=4000], padded to 4096 on host):
  setup:  cast-load ALL of x into SBUF as bf16 ([128=(f4,c), fg, t], 64KB/p).
  pass 1: 1x1-conv+BN+PReLU for q, k (layout [d', t]) and v (layout [t, d']),
          BN folded into conv weights/bias on the host. d' = f*16 + ch,
          f = fg*4 + f4. q/k/v stored bf16.
  pass 2: per t-block of 512: mask_st = sigmoid(k.T q) in 128-row s-chunks
          (flash-style, mask never touches HBM, bf16); out' = sum_s v_s.T
          mask_st accumulated in fp32 PSUM in [d', t] layout; enc conv
          (f32r) + BN + PReLU + residual(bf16 x); store.
The last s-chunk contracts only 32 rows (t in [4000,4096) is padding).
"""
import math

import ml_dtypes
import numpy as np

import concourse.bass as bass
import concourse.mybir as mybir
from concourse import bacc
from concourse import bass_utils
from concourse.tile import TileContext

C, F, T = 32, 32, 4000
CH = 16
D = 512
TB = 512
NTB = 8          # t-blocks (last is ragged: 416 real columns)
NSCH = 32        # s-chunks of 128 (4096 padded)
TPAD = NSCH * 128
EPS = 1e-5

F32 = mybir.dt.float32
F32R = mybir.dt.float32r
BF16 = mybir.dt.bfloat16
AF = mybir.ActivationFunctionType
OP = mybir.AluOpType

TRACE = False
LAST_RESULTS = None


def _build(alpha_q, alpha_k, alpha_v, alpha_e):
    nc = bacc.Bacc("TRN2", target_bir_lowering=False, debug=False)

    x = nc.dram_tensor("x", [C, F, TPAD], BF16, kind="ExternalInput").ap()
    wqA = nc.dram_tensor("wqA", [128, 128], BF16, kind="ExternalInput").ap()
    wqB = nc.dram_tensor("wqB", [128, 128], BF16, kind="ExternalInput").ap()
    wkA = nc.dram_tensor("wkA", [128, 128], BF16, kind="ExternalInput").ap()
    wkB = nc.dram_tensor("wkB", [128, 128], BF16, kind="ExternalInput").ap()
    wv = nc.dram_tensor("wv", [128, 64], BF16, kind="ExternalInput").ap()
    weH = nc.dram_tensor("weH", [128, 128], F32, kind="ExternalInput").ap()
    bq = nc.dram_tensor("bq", [128, 1], F32, kind="ExternalInput").ap()
    bk = nc.dram_tensor("bk", [128, 1], F32, kind="ExternalInput").ap()
    be = nc.dram_tensor("be", [128, 1], F32, kind="ExternalInput").ap()
    bvb = nc.dram_tensor("bvb", [128, 512], F32, kind="ExternalInput").ap()
    y = nc.dram_tensor("y", [C, F, T], F32, kind="ExternalOutput").ap()

    # flat APs for partition p = f4*32 + c at f = fg*4 + f4 (split-partition
    # rearrange APs scramble/crash the HW DMA path; manual APs are exact)
    def x_ap(fg):
        return bass.AP(tensor=x.tensor, offset=fg * 4 * TPAD,
                       ap=[[TPAD, 4], [F * TPAD, 32], [1, TPAD]])

    def y_ap(fg, t0, tw):
        return bass.AP(tensor=y.tensor, offset=fg * 4 * T + t0,
                       ap=[[T, 4], [F * T, 32], [1, tw]])

    with TileContext(nc) as tc:
        with (
            tc.tile_pool(name="persist", bufs=1) as persist,
            tc.tile_pool(name="sapool", bufs=2) as sapool,
            tc.tile_pool(name="mpool", bufs=4) as mpool,
            tc.tile_pool(name="opool", bufs=4) as opool,
            tc.tile_pool(name="pacc", bufs=1, space="PSUM") as pacc,
            tc.tile_pool(name="pmask", bufs=2, space="PSUM") as pmask,
            tc.tile_pool(name="pmisc", bufs=2, space="PSUM") as pmisc,
        ):
            # resident tensors: x (bf16, one tile per fg), q/k in [d', t],
            # v in [t, d']
            xfull = [persist.tile([128, TPAD], BF16, tag=f"xfull{fg}",
                                  name=f"xfull{fg}") for fg in range(8)]
            qT = persist.tile([128, 4, TPAD], BF16, tag="qT")
            kT = persist.tile([128, 4, TPAD], BF16, tag="kT")
            vS = persist.tile([128, NSCH, D], BF16, tag="vS")

            wqA_sb = persist.tile([128, 128], BF16, tag="wqA")
            wqB_sb = persist.tile([128, 128], BF16, tag="wqB")
            wkA_sb = persist.tile([128, 128], BF16, tag="wkA")
            wkB_sb = persist.tile([128, 128], BF16, tag="wkB")
            wv_sb = persist.tile([128, 64], BF16, tag="wv")
            weH_sb = persist.tile([128, 128], F32R, tag="weH")
            nc.sync.dma_start(out=wqA_sb, in_=wqA)
            nc.sync.dma_start(out=wqB_sb, in_=wqB)
            nc.sync.dma_start(out=wkA_sb, in_=wkA)
            nc.sync.dma_start(out=wkB_sb, in_=wkB)
            nc.sync.dma_start(out=wv_sb, in_=wv)
            nc.gpsimd.dma_start(out=weH_sb, in_=weH)
            bq_sb = persist.tile([128, 1], F32, tag="bq")
            bk_sb = persist.tile([128, 1], F32, tag="bk")
            be_sb = persist.tile([128, 1], F32, tag="be")
            bvb_sb = persist.tile([128, 512], F32, tag="bvb")
            nc.sync.dma_start(out=bq_sb, in_=bq)
            nc.sync.dma_start(out=bk_sb, in_=bk)
            nc.sync.dma_start(out=be_sb, in_=be)
            nc.sync.dma_start(out=bvb_sb, in_=bvb)

            # load all of x (bf16, pre-cast on host), one DMA per fg
            for fg in range(8):
                nc.sync.dma_start(out=xfull[fg], in_=x_ap(fg))

            # ---------------- pass 1: q, k, v preproc ----------------
            for tb in range(NTB):
                t0 = tb * TB

                # q/k: psum [128=(h,f4,ch), t] per d'-chunk g, fg = 2g+h
                for dst, wA, wB, bias, alpha in (
                    (kT, wkA_sb, wkB_sb, bk_sb, alpha_k),
                    (qT, wqA_sb, wqB_sb, bq_sb, alpha_q),
                ):
                    for g in range(4):
                        pp = pmisc.tile([128, TB], F32, tag="mm", name="pp")
                        nc.tensor.matmul(pp, wA, xfull[2 * g][:, t0:t0 + TB],
                                         start=True, stop=False)
                        nc.tensor.matmul(pp, wB, xfull[2 * g + 1][:, t0:t0 + TB],
                                         start=False, stop=True)
                        ds = dst[:, g, t0:t0 + TB]
                        nc.scalar.activation(ds, pp, AF.Identity,
                                             bias=bias[:, 0:1], scale=1.0)
                        nc.vector.scalar_tensor_tensor(
                            out=ds, in0=ds, scalar=alpha, in1=ds,
                            op0=OP.mult, op1=OP.max)

                # v: psum [128=t, 512=d'], x slice as stationary
                for i4 in range(4):
                    sch = tb * 4 + i4
                    vp = pmisc.tile([128, 512], F32, tag="mm")
                    for fg in range(8):
                        nc.tensor.matmul(
                            vp[:, fg * 64:(fg + 1) * 64],
                            xfull[fg][:, t0 + i4 * 128:t0 + (i4 + 1) * 128],
                            wv_sb, start=True, stop=True)
                    vs = vS[:, sch, :]
                    nc.vector.tensor_add(vs, vp, bvb_sb)
                    nc.vector.scalar_tensor_tensor(
                        out=vs, in0=vs, scalar=alpha_v, in1=vs,
                        op0=OP.mult, op1=OP.max)

            # ---------------- pass 2: attention + enc ----------------
            def enc_block(sa, tb):
                t0 = tb * TB
                tw = min(TB, T - t0)
                for fg in range(8):
                    g, h = fg // 2, fg % 2
                    ep = pmisc.tile([128, TB], F32, tag="mm", name="ep")
                    nc.tensor.matmul(
                        ep, weH_sb[64 * h:64 * (h + 1), :],
                        sa[64 * h:64 * (h + 1), g, :], start=True, stop=True)
                    es = opool.tile([128, TB], F32, tag="eb", name="es")
                    nc.scalar.activation(es, ep, AF.Identity,
                                         bias=be_sb[:, 0:1], scale=1.0)
                    nc.vector.scalar_tensor_tensor(
                        out=es, in0=es, scalar=alpha_e, in1=es,
                        op0=OP.mult, op1=OP.max)
                    nc.vector.tensor_add(es, es, xfull[fg][:, t0:t0 + TB])
                    nc.sync.dma_start(out=y_ap(fg, t0, tw), in_=es[:, 0:tw])

            for tb in range(NTB):
                t0 = tb * TB

                acc = [pacc.tile([128, TB], F32, tag=f"acc{g}", name=f"acc{g}")
                       for g in range(4)]
                for sch in range(NSCH):
                    mp = pmask.tile([128, TB], F32, tag="mp")
                    for g in range(4):
                        nc.tensor.matmul(
                            mp, kT[:, g, sch * 128:(sch + 1) * 128],
                            qT[:, g, t0:t0 + TB], start=(g == 0), stop=(g == 3))
                    ms = mpool.tile([128, TB], BF16, tag="ms")
                    nc.scalar.activation(ms, mp, AF.Sigmoid)
                    kk = 128 - (TPAD - T) if sch == NSCH - 1 else 128
                    for g in range(4):
                        nc.tensor.matmul(
                            acc[g], vS[0:kk, sch, g * 128:(g + 1) * 128],
                            ms[0:kk, :], start=(sch == 0), stop=(sch == NSCH - 1))

                # psum -> sbuf (f32r) for enc conv
                sa = sapool.tile([128, 4, TB], F32R, tag="sa")
                for g in range(4):
                    nc.vector.tensor_copy(sa[:, g, :], acc[g])

                enc_block(sa, tb)

    nc.finalize()
    return nc


def _host_prep(inputs):
    """Fold BN into conv weights/bias; build block-diagonal weight layouts."""
    qkv_W = np.asarray(inputs["qkv_W"], np.float64)      # [3, 16, 32]
    qkv_b = np.asarray(inputs["qkv_b"], np.float64)      # [3, 16]
    qkv_gamma = np.asarray(inputs["qkv_gamma"], np.float64)
    qkv_beta = np.asarray(inputs["qkv_beta"], np.float64)
    qkv_mean = np.asarray(inputs["qkv_mean"], np.float64)
    qkv_var = np.asarray(inputs["qkv_var"], np.float64)
    qkv_alpha = np.asarray(inputs["qkv_alpha"], np.float64)  # [3]
    enc_W = np.asarray(inputs["enc_W"], np.float64)      # [32, 16]
    enc_b = np.asarray(inputs["enc_b"], np.float64)      # [32]
    enc_gamma = np.asarray(inputs["enc_gamma"], np.float64)
    enc_beta = np.asarray(inputs["enc_beta"], np.float64)
    enc_mean = np.asarray(inputs["enc_mean"], np.float64)
    enc_var = np.asarray(inputs["enc_var"], np.float64)
    enc_alpha = float(np.asarray(inputs["enc_alpha"]))

    inv = qkv_gamma / np.sqrt(qkv_var + EPS)             # [3, 16]
    W_eff = qkv_W * inv[:, :, None]                      # [3, 16, 32]
    b_eff = (qkv_b - qkv_mean) * inv + qkv_beta          # [3, 16]
    scale = math.sqrt(D)
    W_eff[1] /= scale
    b_eff[1] /= scale

    einv = enc_gamma / np.sqrt(enc_var + EPS)            # [32]
    We_eff = enc_W * einv[:, None]                       # [32, 16]
    be_eff = (enc_b - enc_mean) * einv + enc_beta        # [32]

    def blk(Wn):  # [16, 32] -> [128, 64] block-diag over f4
        out = np.zeros((128, 64), np.float32)
        for f4 in range(4):
            out[f4 * 32:(f4 + 1) * 32, f4 * 16:(f4 + 1) * 16] = Wn.T
        return out

    def blkAB(Wn):
        b = blk(Wn)
        A = np.zeros((128, 128), np.float32)
        Bm = np.zeros((128, 128), np.float32)
        A[:, 0:64] = b
        Bm[:, 64:128] = b
        return A, Bm

    wqA, wqB = blkAB(W_eff[0])
    wkA, wkB = blkAB(W_eff[1])
    wv = blk(W_eff[2])

    we_blk = np.zeros((64, 128), np.float32)             # [(f4,ch), (f4,o)]
    for f4 in range(4):
        we_blk[f4 * 16:(f4 + 1) * 16, f4 * 32:(f4 + 1) * 32] = We_eff.T
    weH = np.concatenate([we_blk, we_blk], axis=0)       # [128, 128]

    p = np.arange(128)
    bq = b_eff[0][p % 16].astype(np.float32)[:, None]    # [128, 1]
    bk = b_eff[1][p % 16].astype(np.float32)[:, None]
    be = be_eff[p % 32].astype(np.float32)[:, None]
    cols = np.arange(512)
    bvb = np.broadcast_to(
        b_eff[2][cols % 16].astype(np.float32), (128, 512)).copy()

    alphas = (float(qkv_alpha[0]), float(qkv_alpha[1]),
              float(qkv_alpha[2]), enc_alpha)
    for a in alphas:
        assert 0.0 <= a <= 1.0
    weights = {
        "wqA": wqA, "wqB": wqB, "wkA": wkA, "wkB": wkB, "wv": wv,
        "weH": weH, "bq": bq, "bk": bk, "be": be, "bvb": bvb,
    }
    return weights, alphas


def kernel(**inputs):
    global LAST_RESULTS
    x = np.ascontiguousarray(np.asarray(inputs["x"], np.float32))  # [8,32,32,4000]
    B = x.shape[0]
    x = np.pad(x, ((0, 0), (0, 0), (0, 0), (0, TPAD - T)))
    x = x.astype(ml_dtypes.bfloat16)
    weights, alphas = _host_prep(inputs)
    for k in ("wqA", "wqB", "wkA", "wkB", "wv"):
        weights[k] = weights[k].astype(ml_dtypes.bfloat16)
    nc = _build(*alphas)
    in_maps = [dict(weights, x=x[b]) for b in range(B)]
    res = bass_utils.run_bass_kernel_spmd(
        nc, in_maps, core_ids=list(range(B)), trace=TRACE)
    LAST_RESULTS = res
    out = np.stack([res.results[b]["y"] for b in range(B)], axis=0)
    return out.astype(np.float32)
